# revision 5
# baseline (speedup 1.0000x reference)
"""
BasicCrossAttention Trainium2 kernel (8 NeuronCores, SPMD head-parallel).

Sharding: 16 heads split across 8 cores (2 heads/core).  Each core computes
Q/K/V projections for its 2 heads (column-sharded Wq/Wk/Wv), per-head QK
LayerNorm, full cross-attention over B*H_local, and a row-sharded partial of
the output projection.  The host sums the 8 partial outputs and adds bias.

Math on device is bf16 (matmuls) with fp32 PSUM accumulation; softmax is
exp-without-max (scores are bounded by mup scale 8/64 * |q||k| <= 8) with the
normalizer obtained for free by augmenting V with a ones column.
"""

import os
import sys

for _p in ("/root/.axon_site", "/root/.axon_site/_ro/trn_rl_repo",
           "/root/.axon_site/_ro/pypackages", "/opt/trn_rl_repo"):
    if os.path.isdir(_p) and _p not in sys.path:
        sys.path.append(_p)

import numpy as np
import ml_dtypes
from contextlib import ExitStack

B = 2
N = 2048          # query rows (x1)
M = 2048          # key rows (x2)
DM = 1024         # d_model
H = 16            # total heads
HD = 64           # head dim
NCORES = 8
HL = H // NCORES  # heads per core = 2
LOC = HL * HD     # local feature width = 128
SCALE = 8.0 / HD  # mup scale
EPS = 1e-5

_COMPILED = None          # cached Bass program
LAST_RESULT = None        # BassKernelResults of last run (for profiling)


def _emit(ctx, tc, aps):
    import concourse.bass as bass
    from concourse import mybir
    from concourse.masks import make_identity

    nc = tc.nc
    f32 = mybir.dt.float32
    bf16 = mybir.dt.bfloat16
    AF = mybir.ActivationFunctionType
    OP = mybir.AluOpType

    x1, x2, wqT, wkT, wvT, wp, ln_g, ln_b, out = (
        aps["x1"], aps["x2"], aps["wqT"], aps["wkT"], aps["wvT"],
        aps["wp"], aps["ln_g"], aps["ln_b"], aps["out"])

    const = ctx.enter_context(tc.tile_pool(name="const", bufs=1))
    xT_pool = ctx.enter_context(tc.tile_pool(name="xTp", bufs=2))
    nat_pool = ctx.enter_context(tc.tile_pool(name="natp", bufs=3))
    stat_pool = ctx.enter_context(tc.tile_pool(name="statp", bufs=4))
    big_pool = ctx.enter_context(tc.tile_pool(name="bigp", bufs=2))
    pT_pool = ctx.enter_context(tc.tile_pool(name="pTp", bufs=3))
    out_pool = ctx.enter_context(tc.tile_pool(name="outp", bufs=3))
    bc_pool = ctx.enter_context(tc.tile_pool(name="bcp", bufs=2))
    ps128 = ctx.enter_context(tc.tile_pool(name="ps128", bufs=2, space="PSUM"))
    ps512 = ctx.enter_context(tc.tile_pool(name="ps512", bufs=2, space="PSUM"))

    # ---- constants ----
    ident = const.tile([128, 128], bf16)
    make_identity(nc, ident)

    wq_sb = const.tile([128, 8, LOC], bf16)
    nc.gpsimd.dma_start(wq_sb, wqT.rearrange("(a p) o -> p a o", p=128))
    wk_sb = const.tile([128, 8, LOC], bf16)
    nc.gpsimd.dma_start(wk_sb, wkT.rearrange("(a p) o -> p a o", p=128))
    wv_sb = const.tile([128, 8, LOC], bf16)
    nc.gpsimd.dma_start(wv_sb, wvT.rearrange("(a p) o -> p a o", p=128))
    wp_sb = const.tile([128, DM], bf16)
    nc.gpsimd.dma_start(wp_sb, wp)

    # ln params replicated per local feature: partition p <- param[p % 64]
    g_col = const.tile([128, 1], f32)
    nc.gpsimd.dma_start(g_col, bass.AP(tensor=ln_g.tensor, offset=ln_g.offset,
                                       ap=[[0, HL], [1, HD]]))
    b_col = const.tile([128, 1], f32)
    nc.gpsimd.dma_start(b_col, bass.AP(tensor=ln_b.tensor, offset=ln_b.offset,
                                       ap=[[0, HL], [1, HD]]))
    gq_col = const.tile([128, 1], f32)
    nc.vector.tensor_scalar_mul(gq_col, g_col, SCALE)
    bq_col = const.tile([128, 1], f32)
    nc.vector.tensor_scalar_mul(bq_col, b_col, SCALE)
    eps_col = const.tile([128, 1], f32)
    nc.vector.memset(eps_col, EPS)

    for b in range(B):
        kT_b = big_pool.tile([128, M], bf16, tag="kT")
        qT_b = big_pool.tile([128, N], bf16, tag="qT")
        V_b = big_pool.tile([128, 16, 2 * (HD + 1)], bf16, tag="V")
        hoT_b = big_pool.tile([128, N], bf16, tag="hoT")

        # ---- phase A/B: projections + LN + transposes ----
        # (src=x2 -> K,V ; src=x1 -> Q)
        for src, is_q in ((x2, False), (x1, True)):
            for rg in range(4):  # 512-row groups
                xT = xT_pool.tile([128, 8, 512], bf16, tag="xT")
                for fc in range(8):
                    nc.sync.dma_start(
                        out=xT[:, fc, :],
                        in_=src[b, rg * 512:(rg + 1) * 512,
                                fc * 128:(fc + 1) * 128],
                        transpose=True)
                for mi in range(4):
                    mt = rg * 4 + mi       # global 128-row tile index
                    rs = slice(mi * 128, (mi + 1) * 128)
                    w_sb = wq_sb if is_q else wk_sb
                    ps = ps128.tile([128, 128], f32, tag="ps128")
                    for fc in range(8):
                        nc.tensor.matmul(ps, lhsT=xT[:, fc, rs],
                                         rhs=w_sb[:, fc, :],
                                         start=(fc == 0), stop=(fc == 7))
                    # per-head layernorm stats over free dim (64 per head)
                    stats = stat_pool.tile([128, HL, 6], f32, tag="stats")
                    mv = stat_pool.tile([128, HL, 2], f32, tag="mv")
                    for h in range(HL):
                        nc.vector.bn_stats(stats[:, h, :],
                                           ps[:, h * HD:(h + 1) * HD])
                        nc.vector.bn_aggr(mv[:, h, :], stats[:, h, :])
                    rstd = stat_pool.tile([128, HL], f32, tag="rstd")
                    nc.scalar.activation(rstd, mv[:, :, 1], AF.Sqrt,
                                         bias=eps_col)
                    nc.vector.reciprocal(rstd, rstd)
                    nmean = stat_pool.tile([128, HL], f32, tag="nmean")
                    nc.vector.tensor_scalar_mul(nmean, mv[:, :, 0], -1.0)
                    nb = stat_pool.tile([128, HL], f32, tag="nb")
                    nc.vector.tensor_mul(nb, nmean, rstd)
                    qk_nat = nat_pool.tile([128, 128], bf16, tag="nat")
                    for h in range(HL):
                        hs = slice(h * HD, (h + 1) * HD)
                        nc.vector.tensor_scalar(
                            qk_nat[:, hs], ps[:, hs],
                            rstd[:, h:h + 1], nb[:, h:h + 1],
                            op0=OP.mult, op1=OP.add)
                    tps = ps128.tile([128, 128], bf16, tag="trps")
                    nc.tensor.transpose(tps, qk_nat, ident)
                    dst = qT_b if is_q else kT_b
                    gc, bc = (gq_col, bq_col) if is_q else (g_col, b_col)
                    nc.vector.tensor_scalar(
                        dst[:, mt * 128:(mt + 1) * 128], tps, gc, bc,
                        op0=OP.mult, op1=OP.add)

                    if not is_q:
                        # V projection (natural layout) + ones columns
                        psv = ps128.tile([128, 128], f32, tag="ps128")
                        for fc in range(8):
                            nc.tensor.matmul(psv, lhsT=xT[:, fc, rs],
                                             rhs=wv_sb[:, fc, :],
                                             start=(fc == 0), stop=(fc == 7))
                        vt = V_b[:, mt, :]
                        nc.vector.memset(vt[:, HD::HD + 1], 1.0)
                        for h in range(HL):
                            nc.vector.tensor_copy(
                                vt[:, h * (HD + 1):h * (HD + 1) + HD],
                                psv[:, h * HD:(h + 1) * HD])

        # ---- phase C: attention (S^T = k@q^T -> exp -> (V|1)^T @ P^T) ----
        for h in range(HL):
            hs = slice(h * HD, (h + 1) * HD)
            vs = slice(h * (HD + 1), (h + 1) * (HD + 1))
            for nc4 in range(4):  # 512-wide query column chunks
                ns = slice(nc4 * 512, (nc4 + 1) * 512)
                avps = ps512.tile([128, 512], f32, tag="av")
                for mc in range(16):
                    stps = ps512.tile([128, 512], f32, tag="st")
                    nc.tensor.matmul(stps,
                                     lhsT=kT_b[hs, mc * 128:(mc + 1) * 128],
                                     rhs=qT_b[hs, ns],
                                     start=True, stop=True)
                    pT = pT_pool.tile([128, 512], bf16, tag="pT")
                    nc.scalar.activation(pT, stps, AF.Exp)
                    nc.tensor.matmul(avps[0:HD + 1, :],
                                     lhsT=V_b[:, mc, vs], rhs=pT,
                                     start=(mc == 0), stop=(mc == 15),
                                     skip_group_check=True)
                recip = bc_pool.tile([1, 512], f32, tag="recip")
                nc.vector.reciprocal(recip, avps[HD:HD + 1, :])
                bcast = bc_pool.tile([HD, 512], f32, tag="bcast")
                nc.gpsimd.partition_broadcast(bcast, recip)
                nc.vector.tensor_mul(hoT_b[hs, ns], avps[0:HD, :], bcast)

        # ---- phase D: output projection partial ----
        for nt in range(16):
            for oc in range(2):
                fps = ps512.tile([128, 512], f32, tag="st")
                nc.tensor.matmul(fps,
                                 lhsT=hoT_b[:, nt * 128:(nt + 1) * 128],
                                 rhs=wp_sb[:, oc * 512:(oc + 1) * 512],
                                 start=True, stop=True)
                osb = out_pool.tile([128, 512], f32, tag="osb")
                nc.vector.tensor_copy(osb, fps)
                nc.sync.dma_start(
                    out[b, nt * 128:(nt + 1) * 128, oc * 512:(oc + 1) * 512],
                    osb)


def _build():
    global _COMPILED
    if _COMPILED is not None:
        return _COMPILED
    import concourse.tile as tile
    from concourse import bacc, mybir

    nc = bacc.Bacc("TRN2", target_bir_lowering=False, debug=False,
                   enable_asserts=False)
    bf16 = mybir.dt.bfloat16
    f32 = mybir.dt.float32
    aps = {
        "x1": nc.dram_tensor("x1", [B, N, DM], bf16, kind="ExternalInput").ap(),
        "x2": nc.dram_tensor("x2", [B, M, DM], bf16, kind="ExternalInput").ap(),
        "wqT": nc.dram_tensor("wqT", [DM, LOC], bf16, kind="ExternalInput").ap(),
        "wkT": nc.dram_tensor("wkT", [DM, LOC], bf16, kind="ExternalInput").ap(),
        "wvT": nc.dram_tensor("wvT", [DM, LOC], bf16, kind="ExternalInput").ap(),
        "wp": nc.dram_tensor("wp", [LOC, DM], bf16, kind="ExternalInput").ap(),
        "ln_g": nc.dram_tensor("ln_g", [HD], f32, kind="ExternalInput").ap(),
        "ln_b": nc.dram_tensor("ln_b", [HD], f32, kind="ExternalInput").ap(),
        "out": nc.dram_tensor("out", [B, N, DM], f32, kind="ExternalOutput").ap(),
    }
    with tile.TileContext(nc) as tc, ExitStack() as ctx:
        _emit(ctx, tc, aps)
    nc.compile()
    _COMPILED = nc
    return nc


def kernel(x1, x2, Wq, Wk, Wv, Wp, bp, ln_g, ln_b):
    global LAST_RESULT
    from concourse.bass_utils import run_bass_kernel_spmd

    nc = _build()
    bf = ml_dtypes.bfloat16
    x1b = np.ascontiguousarray(np.asarray(x1, dtype=np.float32)).astype(bf)
    x2b = np.ascontiguousarray(np.asarray(x2, dtype=np.float32)).astype(bf)
    Wq = np.asarray(Wq, dtype=np.float32)
    Wk = np.asarray(Wk, dtype=np.float32)
    Wv = np.asarray(Wv, dtype=np.float32)
    Wp = np.asarray(Wp, dtype=np.float32)
    ln_g32 = np.ascontiguousarray(np.asarray(ln_g, dtype=np.float32))
    ln_b32 = np.ascontiguousarray(np.asarray(ln_b, dtype=np.float32))

    in_maps = []
    for c in range(NCORES):
        hs = slice(c * LOC, (c + 1) * LOC)
        in_maps.append({
            "x1": x1b,
            "x2": x2b,
            "wqT": np.ascontiguousarray(Wq[hs, :].T).astype(bf),
            "wkT": np.ascontiguousarray(Wk[hs, :].T).astype(bf),
            "wvT": np.ascontiguousarray(Wv[hs, :].T).astype(bf),
            "wp": np.ascontiguousarray(Wp[:, hs].T).astype(bf),
            "ln_g": ln_g32,
            "ln_b": ln_b32,
        })

    res = run_bass_kernel_spmd(nc, in_maps, core_ids=list(range(NCORES)))
    LAST_RESULT = res
    acc = np.zeros((B, N, DM), dtype=np.float32)
    for r in res.results:
        acc += r["out"]
    acc += np.asarray(bp, dtype=np.float32)
    return acc


# revision 9
# speedup vs baseline: 1.3555x; 1.3555x over previous
"""
BasicCrossAttention Trainium2 kernel (8 NeuronCores, SPMD head-parallel).

Sharding: 16 heads split across 8 cores (2 heads/core).  Each core computes
Q/K/V projections for its 2 heads (column-sharded Wq/Wk/Wv), per-head QK
LayerNorm, full cross-attention over B*H_local, and a row-sharded partial of
the output projection.  The host sums the 8 partial outputs and adds bias.

Device math is bf16 matmuls with fp32 PSUM accumulation.

Structure (per core):
  - x1/x2 arrive bf16; xT tiles produced by DMA-xbar transpose (DRAM->SBUF).
  - K|V projected in one N=256 matmul group; Q in an N=128 group.
  - QK LayerNorm: K/Q weight columns are mean-centered per head on-device at
    startup, so projections yield zero-mean heads directly; only E[x^2] is
    needed (one square + one 3D reduce per tile).  rstd = exp(-0.5*ln(var+eps))
    batched per row-group on ACT -- keeps ACT on one table set (ln+exp), no
    table thrashing with softmax's exp.
  - Attention in S^T layout [m,n]: head-pair row-packed score matmuls (K=64
    at row groups 0/64) into one [128,1024] PSUM pair, one paired exp, and
    V-augmented-with-ones AV matmuls giving the softmax normalizer for free.
  - Output projection partials drain via nc.any copies.
Emission interleaves attention(b) with projection(b+1) / output(b-1) so the
PE and DMA streams stay dense while ACT (softmax exp) is the paced engine.
"""

import os
import sys

for _p in ("/root/.axon_site", "/root/.axon_site/_ro/trn_rl_repo",
           "/root/.axon_site/_ro/pypackages", "/opt/trn_rl_repo"):
    if os.path.isdir(_p) and _p not in sys.path:
        sys.path.append(_p)

import numpy as np
import ml_dtypes
from contextlib import ExitStack

B = 2
N = 2048          # query rows (x1)
M = 2048          # key rows (x2)
DM = 1024         # d_model
H = 16            # total heads
HD = 64           # head dim
NCORES = 8
HL = H // NCORES  # heads per core = 2
LOC = HL * HD     # local feature width = 128
SCALE = 8.0 / HD  # mup scale
EPS = 1e-5

_COMPILED = None          # cached Bass program
LAST_RESULT = None        # BassKernelResults of last run (for profiling)


def _emit(ctx, tc, aps):
    import concourse.bass as bass
    from concourse import mybir
    from concourse.masks import make_identity

    nc = tc.nc
    f32 = mybir.dt.float32
    bf16 = mybir.dt.bfloat16
    AF = mybir.ActivationFunctionType
    OP = mybir.AluOpType

    x1, x2, wqT, wkT, wvT, wp, ln_g, ln_b, out = (
        aps["x1"], aps["x2"], aps["wqT"], aps["wkT"], aps["wvT"],
        aps["wp"], aps["ln_g"], aps["ln_b"], aps["out"])

    const = ctx.enter_context(tc.tile_pool(name="const", bufs=1))
    xT_pool = ctx.enter_context(tc.tile_pool(name="xTp", bufs=2))
    nat_pool = ctx.enter_context(tc.tile_pool(name="natp", bufs=10))
    stat_pool = ctx.enter_context(tc.tile_pool(name="statp", bufs=3))
    big_pool = ctx.enter_context(tc.tile_pool(name="bigp", bufs=2))
    pT_pool = ctx.enter_context(tc.tile_pool(name="pTp", bufs=3))
    out_pool = ctx.enter_context(tc.tile_pool(name="outp", bufs=3))
    bc_pool = ctx.enter_context(tc.tile_pool(name="bcp", bufs=2))
    ps128 = ctx.enter_context(tc.tile_pool(name="ps128", bufs=2, space="PSUM"))
    psbig = ctx.enter_context(tc.tile_pool(name="psbig", bufs=2, space="PSUM"))

    # ---------------- constants / weights ----------------
    ident = const.tile([128, 128], bf16)
    make_identity(nc, ident)

    # wkv layout: [in 128, fc 8, k(128) | v(128)]
    wkv_sb = const.tile([128, 8, 2 * LOC], bf16)
    nc.gpsimd.dma_start(wkv_sb[:, :, 0:LOC],
                        wkT.rearrange("(a p) o -> p a o", p=128))
    nc.gpsimd.dma_start(wkv_sb[:, :, LOC:2 * LOC],
                        wvT.rearrange("(a p) o -> p a o", p=128))
    wq_sb = const.tile([128, 8, LOC], bf16)
    nc.gpsimd.dma_start(wq_sb, wqT.rearrange("(a p) o -> p a o", p=128))
    wp_sb = const.tile([128, DM], bf16)
    nc.gpsimd.dma_start(wp_sb, wp)

    # ln params replicated per local feature: partition p <- param[p % 64]
    g_col = const.tile([128, 1], f32)
    nc.gpsimd.dma_start(g_col, bass.AP(tensor=ln_g.tensor, offset=ln_g.offset,
                                       ap=[[0, HL], [1, HD]]))
    b_col = const.tile([128, 1], f32)
    nc.gpsimd.dma_start(b_col, bass.AP(tensor=ln_b.tensor, offset=ln_b.offset,
                                       ap=[[0, HL], [1, HD]]))
    gq_col = const.tile([128, 1], f32)
    nc.vector.tensor_scalar_mul(gq_col, g_col, SCALE)
    bq_col = const.tile([128, 1], f32)
    nc.vector.tensor_scalar_mul(bq_col, b_col, SCALE)
    eps_col = const.tile([128, 1], f32)
    nc.vector.memset(eps_col, EPS)

    # Mean-center the K and Q weight head-blocks so projections are zero-mean
    # per head (LN mean handled in the weights; only E[x^2] needed per tile).
    for w_sb, nblk in ((wkv_sb, HL), (wq_sb, HL)):
        for fc in range(8):
            for h in range(nblk):
                blk = w_sb[:, fc, h * HD:(h + 1) * HD]
                m = stat_pool.tile([128, 1], f32, tag="wm")
                nc.vector.reduce_sum(m, blk, axis=mybir.AxisListType.X)
                nc.vector.tensor_scalar_mul(m, m, 1.0 / HD)
                nc.vector.tensor_scalar(blk, blk, m, None, op0=OP.subtract)

    # persistent per-batch tiles (bufs=2 -> both batches in flight)
    kT = [None, None]
    qT = [None, None]
    Vt = [None, None]
    hoT = [None, None]

    # ---------------- phase generators ----------------
    def prod(b):
        """Project K|V (from x2) and Q (from x1) for batch b; LN; transposes."""
        kT[b] = big_pool.tile([128, M], bf16, tag="kT", name=f"kT{b}")
        qT[b] = big_pool.tile([128, N], bf16, tag="qT", name=f"qT{b}")
        Vt[b] = big_pool.tile([128, 16, 2 * (HD + 1)], bf16, tag="V",
                              name=f"V{b}")
        for src, is_q in ((x2, False), (x1, True)):
            w_sb = wq_sb if is_q else wkv_sb
            nout = LOC if is_q else 2 * LOC
            for rg in range(2):  # 1024-row groups
                xT = xT_pool.tile([128, 8, 1024], bf16, tag="xT",
                                  name=f"xT{b}{int(is_q)}{rg}")
                for fc in range(8):
                    nc.sync.dma_start(
                        out=xT[:, fc, :],
                        in_=src[b, rg * 1024:(rg + 1) * 1024,
                                fc * 128:(fc + 1) * 128],
                        transpose=True)
                yield
                s2g = stat_pool.tile([128, 8, HL], f32, tag="s2g",
                                     name=f"s2g{b}{int(is_q)}{rg}")
                raws = []
                for mi in range(8):
                    mt = rg * 8 + mi  # global 128-row tile index
                    rs = slice(mi * 128, (mi + 1) * 128)
                    ps = ps128.tile([128, nout], f32, tag="ps128",
                                    name=f"ps{b}{int(is_q)}{mt}")
                    for fc in range(8):
                        nc.tensor.matmul(ps, lhsT=xT[:, fc, rs],
                                         rhs=w_sb[:, fc, :],
                                         start=(fc == 0), stop=(fc == 7))
                    raw = nat_pool.tile([128, LOC], bf16, tag="raw", bufs=10,
                                        name=f"raw{b}{int(is_q)}{mt}")
                    nc.vector.tensor_copy(raw, ps[:, 0:LOC])
                    raws.append(raw)
                    # E[x^2] per head for LN (weights are centered)
                    sq = nat_pool.tile([128, LOC], f32, tag="sq", bufs=2)
                    nc.vector.tensor_mul(sq, raw, raw)
                    nc.vector.reduce_sum(s2g[:, mi, :],
                                         sq.rearrange("p (h d) -> p h d", h=HL),
                                         axis=mybir.AxisListType.X)
                    if not is_q:
                        vt = Vt[b][:, mt, :]
                        nc.vector.memset(vt[:, HD::HD + 1], 1.0)
                        for h in range(HL):
                            nc.vector.tensor_copy(
                                vt[:, h * (HD + 1):h * (HD + 1) + HD],
                                ps[:, LOC + h * HD:LOC + (h + 1) * HD])
                    yield
                # batched rstd for the whole row-group (ACT stays on the
                # ln/exp table set shared with softmax exp)
                rstdg = stat_pool.tile([128, 8, HL], f32, tag="rstdg")
                flat_in = s2g.rearrange("p a b -> p (a b)")
                flat_out = rstdg.rearrange("p a b -> p (a b)")
                nc.scalar.activation(flat_out, flat_in, AF.Ln,
                                     bias=eps_col, scale=1.0 / HD)
                nc.scalar.activation(flat_out, flat_out, AF.Exp, scale=-0.5)
                dst = qT[b] if is_q else kT[b]
                gc, bc = (gq_col, bq_col) if is_q else (g_col, b_col)
                for mi in range(8):
                    mt = rg * 8 + mi
                    nrm = nat_pool.tile([128, LOC], bf16, tag="nrm", bufs=3)
                    for h in range(HL):
                        hs = slice(h * HD, (h + 1) * HD)
                        nc.vector.tensor_scalar(
                            nrm[:, hs], raws[mi][:, hs],
                            rstdg[:, mi, h:h + 1], None, op0=OP.mult)
                    tps = ps128.tile([128, 128], bf16, tag="ps128",
                                     name=f"tps{b}{int(is_q)}{mt}")
                    nc.tensor.transpose(tps, nrm, ident)
                    nc.vector.tensor_scalar(
                        dst[:, mt * 128:(mt + 1) * 128], tps, gc, bc,
                        op0=OP.mult, op1=OP.add)
                    yield

    def attn(b):
        """S^T -> exp -> (V|1)^T @ P^T, head-pair packed."""
        hoT[b] = big_pool.tile([128, N], bf16, tag="hoT", name=f"hoT{b}")
        for nc4 in range(4):  # 512-wide query column chunks
            ns = slice(nc4 * 512, (nc4 + 1) * 512)
            av = psbig.tile([128, 1024], f32, tag="av", bufs=1,
                            name=f"av{b}{nc4}")
            for mc in range(16):
                mcs = slice(mc * 128, (mc + 1) * 128)
                st = psbig.tile([128, 1024], f32, tag="st",
                                name=f"st{b}{nc4}{mc}")
                for h in range(HL):
                    nc.tensor.matmul(st[:, h * 512:(h + 1) * 512],
                                     lhsT=kT[b][h * HD:(h + 1) * HD, mcs],
                                     rhs=qT[b][h * HD:(h + 1) * HD, ns],
                                     start=True, stop=True)
                pT = pT_pool.tile([128, 1024], bf16, tag="pT")
                nc.scalar.activation(pT, st, AF.Exp)
                for h in range(HL):
                    nc.tensor.matmul(
                        av[0:HD + 1, h * 512:(h + 1) * 512],
                        lhsT=Vt[b][:, mc, h * (HD + 1):(h + 1) * (HD + 1)],
                        rhs=pT[:, h * 512:(h + 1) * 512],
                        start=(mc == 0), stop=(mc == 15),
                        skip_group_check=True)
                yield
            for h in range(HL):
                hs = slice(h * HD, (h + 1) * HD)
                sl = slice(h * 512, (h + 1) * 512)
                recip = bc_pool.tile([1, 512], f32, tag="recip")
                nc.vector.reciprocal(recip, av[HD:HD + 1, sl])
                bcast = bc_pool.tile([HD, 512], f32, tag="bcast")
                nc.gpsimd.partition_broadcast(bcast, recip)
                nc.vector.tensor_mul(hoT[b][hs, ns], av[0:HD, sl], bcast)
                yield

    def outp(b):
        """Output projection partial for batch b."""
        for nt in range(16):
            for oc in range(2):
                fps = psbig.tile([128, 512], f32, tag="st",
                                 name=f"fps{b}{nt}{oc}")
                nc.tensor.matmul(fps,
                                 lhsT=hoT[b][:, nt * 128:(nt + 1) * 128],
                                 rhs=wp_sb[:, oc * 512:(oc + 1) * 512],
                                 start=True, stop=True)
                osb = out_pool.tile([128, 512], f32, tag="osb")
                nc.any.tensor_copy(osb, fps)
                nc.gpsimd.dma_start(
                    out[b, nt * 128:(nt + 1) * 128, oc * 512:(oc + 1) * 512],
                    osb)
                yield

    def run_all(g):
        for _ in g:
            pass

    def interleave(ga, gb, ka, kb):
        """Alternate ka steps of ga with kb steps of gb until both drain."""
        alive_a, alive_b = True, True
        while alive_a or alive_b:
            for _ in range(ka):
                if alive_a:
                    alive_a = next(ga, _SENTINEL) is not _SENTINEL
            for _ in range(kb):
                if alive_b:
                    alive_b = next(gb, _SENTINEL) is not _SENTINEL

    _SENTINEL = object()

    run_all(prod(0))
    interleave(attn(0), prod(1), 1, 1)
    interleave(attn(1), outp(0), 2, 1)
    run_all(outp(1))


def _build():
    global _COMPILED
    if _COMPILED is not None:
        return _COMPILED
    import concourse.tile as tile
    from concourse import bacc, mybir

    nc = bacc.Bacc("TRN2", target_bir_lowering=False, debug=False,
                   enable_asserts=False)
    bf16 = mybir.dt.bfloat16
    f32 = mybir.dt.float32
    aps = {
        "x1": nc.dram_tensor("x1", [B, N, DM], bf16, kind="ExternalInput").ap(),
        "x2": nc.dram_tensor("x2", [B, M, DM], bf16, kind="ExternalInput").ap(),
        "wqT": nc.dram_tensor("wqT", [DM, LOC], bf16, kind="ExternalInput").ap(),
        "wkT": nc.dram_tensor("wkT", [DM, LOC], bf16, kind="ExternalInput").ap(),
        "wvT": nc.dram_tensor("wvT", [DM, LOC], bf16, kind="ExternalInput").ap(),
        "wp": nc.dram_tensor("wp", [LOC, DM], bf16, kind="ExternalInput").ap(),
        "ln_g": nc.dram_tensor("ln_g", [HD], f32, kind="ExternalInput").ap(),
        "ln_b": nc.dram_tensor("ln_b", [HD], f32, kind="ExternalInput").ap(),
        "out": nc.dram_tensor("out", [B, N, DM], f32, kind="ExternalOutput").ap(),
    }
    with tile.TileContext(nc) as tc, ExitStack() as ctx:
        _emit(ctx, tc, aps)
    nc.compile()
    _COMPILED = nc
    return nc


def kernel(x1, x2, Wq, Wk, Wv, Wp, bp, ln_g, ln_b):
    global LAST_RESULT
    from concourse.bass_utils import run_bass_kernel_spmd

    nc = _build()
    bf = ml_dtypes.bfloat16
    x1b = np.ascontiguousarray(np.asarray(x1, dtype=np.float32)).astype(bf)
    x2b = np.ascontiguousarray(np.asarray(x2, dtype=np.float32)).astype(bf)
    Wq = np.asarray(Wq, dtype=np.float32)
    Wk = np.asarray(Wk, dtype=np.float32)
    Wv = np.asarray(Wv, dtype=np.float32)
    Wp = np.asarray(Wp, dtype=np.float32)
    ln_g32 = np.ascontiguousarray(np.asarray(ln_g, dtype=np.float32))
    ln_b32 = np.ascontiguousarray(np.asarray(ln_b, dtype=np.float32))

    in_maps = []
    for c in range(NCORES):
        hs = slice(c * LOC, (c + 1) * LOC)
        in_maps.append({
            "x1": x1b,
            "x2": x2b,
            "wqT": np.ascontiguousarray(Wq[hs, :].T).astype(bf),
            "wkT": np.ascontiguousarray(Wk[hs, :].T).astype(bf),
            "wvT": np.ascontiguousarray(Wv[hs, :].T).astype(bf),
            "wp": np.ascontiguousarray(Wp[:, hs].T).astype(bf),
            "ln_g": ln_g32,
            "ln_b": ln_b32,
        })

    res = run_bass_kernel_spmd(nc, in_maps, core_ids=list(range(NCORES)))
    LAST_RESULT = res
    acc = np.zeros((B, N, DM), dtype=np.float32)
    for r in res.results:
        acc += r["out"]
    acc += np.asarray(bp, dtype=np.float32)
    return acc


# revision 16
# speedup vs baseline: 1.4429x; 1.0645x over previous
"""
BasicCrossAttention Trainium2 kernel (8 NeuronCores, SPMD head-parallel).

Sharding: 16 heads split across 8 cores (2 heads/core).  Each core computes
Q/K/V projections for its 2 heads (column-sharded Wq/Wk/Wv), per-head QK
LayerNorm, full cross-attention over B*H_local, and a row-sharded partial of
the output projection.  The host sums the 8 partial outputs and adds bias.

Device math is bf16 matmuls with fp32 PSUM accumulation.

Structure (per core):
  - x1/x2 arrive bf16; xT tiles produced by DMA-xbar transpose (DRAM->SBUF).
  - K|V projected in one N=256 matmul group; Q in an N=128 group.
  - QK LayerNorm: K/Q weight columns are mean-centered per head on-device at
    startup, so projections yield zero-mean heads directly; only E[x^2] is
    needed (one square + one 3D reduce per tile).  rstd = exp(-0.5*ln(var+eps))
    batched per row-group on ACT -- keeps ACT on one table set (ln+exp), no
    table thrashing with softmax's exp.
  - Attention in S^T layout [m,n]: head-pair row-packed score matmuls (K=64
    at row groups 0/64) into one [128,1024] PSUM pair, one paired exp, and
    V-augmented-with-ones AV matmuls giving the softmax normalizer for free.
  - Output projection partials drain via nc.any copies.
Emission interleaves attention(b) with projection(b+1) / output(b-1) so the
PE and DMA streams stay dense while ACT (softmax exp) is the paced engine.
"""

import os
import sys

for _p in ("/root/.axon_site", "/root/.axon_site/_ro/trn_rl_repo",
           "/root/.axon_site/_ro/pypackages", "/opt/trn_rl_repo"):
    if os.path.isdir(_p) and _p not in sys.path:
        sys.path.append(_p)

import numpy as np
import ml_dtypes
from contextlib import ExitStack

B = 2
N = 2048          # query rows (x1)
M = 2048          # key rows (x2)
DM = 1024         # d_model
H = 16            # total heads
HD = 64           # head dim
NCORES = 8
HL = H // NCORES  # heads per core = 2
LOC = HL * HD     # local feature width = 128
SCALE = 8.0 / HD  # mup scale
EPS = 1e-5

_COMPILED = None          # cached Bass program
LAST_RESULT = None        # BassKernelResults of last run (for profiling)


def _emit(ctx, tc, aps):
    import concourse.bass as bass
    from concourse import mybir
    from concourse.masks import make_identity

    nc = tc.nc
    f32 = mybir.dt.float32
    bf16 = mybir.dt.bfloat16
    AF = mybir.ActivationFunctionType
    OP = mybir.AluOpType

    x1, x2, wqT, wkT, wvT, wp, ln_g, ln_b, out = (
        aps["x1"], aps["x2"], aps["wqT"], aps["wkT"], aps["wvT"],
        aps["wp"], aps["ln_g"], aps["ln_b"], aps["out"])

    const = ctx.enter_context(tc.tile_pool(name="const", bufs=1))
    xT_pool = ctx.enter_context(tc.tile_pool(name="xTp", bufs=2))
    nat_pool = ctx.enter_context(tc.tile_pool(name="natp", bufs=10))
    stat_pool = ctx.enter_context(tc.tile_pool(name="statp", bufs=3))
    big_pool = ctx.enter_context(tc.tile_pool(name="bigp", bufs=2))
    pT_pool = ctx.enter_context(tc.tile_pool(name="pTp", bufs=3))
    out_pool = ctx.enter_context(tc.tile_pool(name="outp", bufs=3))
    bc_pool = ctx.enter_context(tc.tile_pool(name="bcp", bufs=2))
    ps128 = ctx.enter_context(tc.tile_pool(name="ps128", bufs=2, space="PSUM"))
    psbig = ctx.enter_context(tc.tile_pool(name="psbig", bufs=2, space="PSUM"))

    # ---------------- constants / weights ----------------
    ident = const.tile([128, 128], bf16)
    make_identity(nc, ident)

    # wkv layout: [in 128, fc 8, k(128) | v(128)]
    wkv_sb = const.tile([128, 8, 2 * LOC], bf16)
    nc.gpsimd.dma_start(wkv_sb[:, :, 0:LOC],
                        wkT.rearrange("(a p) o -> p a o", p=128))
    nc.gpsimd.dma_start(wkv_sb[:, :, LOC:2 * LOC],
                        wvT.rearrange("(a p) o -> p a o", p=128))
    wq_sb = const.tile([128, 8, LOC], bf16)
    nc.gpsimd.dma_start(wq_sb, wqT.rearrange("(a p) o -> p a o", p=128))
    wp_sb = const.tile([128, DM], bf16)
    nc.gpsimd.dma_start(wp_sb, wp)

    # ln params replicated per local feature: partition p <- param[p % 64]
    g_col = const.tile([128, 1], f32)
    nc.gpsimd.dma_start(g_col, bass.AP(tensor=ln_g.tensor, offset=ln_g.offset,
                                       ap=[[0, HL], [1, HD]]))
    b_col = const.tile([128, 1], f32)
    nc.gpsimd.dma_start(b_col, bass.AP(tensor=ln_b.tensor, offset=ln_b.offset,
                                       ap=[[0, HL], [1, HD]]))
    gq_col = const.tile([128, 1], f32)
    nc.vector.tensor_scalar_mul(gq_col, g_col, SCALE)
    bq_col = const.tile([128, 1], f32)
    nc.vector.tensor_scalar_mul(bq_col, b_col, SCALE)

    # Mean-center the K and Q weight head-blocks so projections are zero-mean
    # per head (LN mean handled in the weights; only E[x^2] needed per tile).
    for w_sb, nblk in ((wkv_sb, HL), (wq_sb, HL)):
        for fc in range(8):
            for h in range(nblk):
                blk = w_sb[:, fc, h * HD:(h + 1) * HD]
                m = stat_pool.tile([128, 1], f32, tag="wm")
                nc.vector.reduce_sum(m, blk, axis=mybir.AxisListType.X)
                nc.vector.tensor_scalar_mul(m, m, 1.0 / HD)
                nc.vector.tensor_scalar(blk, blk, m, None, op0=OP.subtract)

    # persistent per-batch tiles (bufs=2 -> both batches in flight)
    kT = [None, None]
    qT = [None, None]
    Vt = [None, None]
    hoT = [None, None]

    # ---------------- phase generators ----------------
    def prod(b):
        """Project K|V (from x2) and Q (from x1) for batch b; LN; transposes."""
        kT[b] = big_pool.tile([128, M], bf16, tag="kT", name=f"kT{b}")
        qT[b] = big_pool.tile([128, N], bf16, tag="qT", name=f"qT{b}")
        Vt[b] = big_pool.tile([128, 16, 2 * (HD + 1)], bf16, tag="V",
                              name=f"V{b}")
        for src, is_q in ((x2, False), (x1, True)):
            w_sb = wq_sb if is_q else wkv_sb
            nout = LOC if is_q else 2 * LOC
            for rg in range(2):  # 1024-row groups
                xT = xT_pool.tile([128, 8, 1024], bf16, tag="xT",
                                  name=f"xT{b}{int(is_q)}{rg}")
                for fc in range(8):
                    nc.sync.dma_start(
                        out=xT[:, fc, :],
                        in_=src[b, rg * 1024:(rg + 1) * 1024,
                                fc * 128:(fc + 1) * 128],
                        transpose=True)
                yield
                s2g = stat_pool.tile([128, 8, HL], f32, tag="s2g",
                                     name=f"s2g{b}{int(is_q)}{rg}")
                raws = []
                for mi in range(8):
                    mt = rg * 8 + mi  # global 128-row tile index
                    rs = slice(mi * 128, (mi + 1) * 128)
                    ps = ps128.tile([128, nout], f32, tag="ps128",
                                    name=f"ps{b}{int(is_q)}{mt}")
                    for fc in range(8):
                        nc.tensor.matmul(ps, lhsT=xT[:, fc, rs],
                                         rhs=w_sb[:, fc, :],
                                         start=(fc == 0), stop=(fc == 7))
                    raw = nat_pool.tile([128, LOC], bf16, tag="raw", bufs=10,
                                        name=f"raw{b}{int(is_q)}{mt}")
                    nc.vector.tensor_copy(raw, ps[:, 0:LOC])
                    raws.append(raw)
                    # E[x^2] per head for LN (weights are centered)
                    sq = nat_pool.tile([128, LOC], f32, tag="sq", bufs=2)
                    nc.vector.tensor_mul(sq, raw, raw)
                    nc.vector.reduce_sum(s2g[:, mi, :],
                                         sq.rearrange("p (h d) -> p h d", h=HL),
                                         axis=mybir.AxisListType.X)
                    if not is_q:
                        vt = Vt[b][:, mt, :]
                        nc.vector.memset(vt[:, HD::HD + 1], 1.0)
                        vt3 = bass.AP(tensor=vt.tensor, offset=vt.offset,
                                      ap=[vt.ap[0], [HD + 1, HL], [1, HD]])
                        nc.vector.tensor_copy(
                            vt3, ps[:, LOC:2 * LOC].rearrange(
                                "p (h x) -> p h x", h=HL))
                    yield
                # batched rstd for the whole row-group, computed on DVE as
                # rsqrt(var+eps) via linear seed + 3 Newton steps (keeps ACT
                # exclusively on softmax exp -- no table-set thrashing)
                rstdg = stat_pool.tile([128, 8, HL], f32, tag="rstdg")
                y = rstdg.rearrange("p a b -> p (a b)")
                var = stat_pool.tile([128, 8 * HL], f32, tag="lnvar")
                tnr = stat_pool.tile([128, 8 * HL], f32, tag="lntnr")
                nc.vector.tensor_scalar(var, s2g.rearrange("p a b -> p (a b)"),
                                        1.0 / HD, EPS, op0=OP.mult, op1=OP.add)
                nc.vector.tensor_scalar(y, var, -0.285, 1.42,
                                        op0=OP.mult, op1=OP.add)
                for _ in range(3):
                    nc.vector.tensor_mul(tnr, y, y)
                    nc.vector.tensor_mul(tnr, tnr, var)
                    nc.vector.tensor_scalar(tnr, tnr, -0.5, 1.5,
                                            op0=OP.mult, op1=OP.add)
                    nc.vector.tensor_mul(y, y, tnr)
                dst = qT[b] if is_q else kT[b]
                gc, bc = (gq_col, bq_col) if is_q else (g_col, b_col)
                for mi in range(8):
                    mt = rg * 8 + mi
                    nrm = nat_pool.tile([128, LOC], bf16, tag="nrm", bufs=3)
                    for h in range(HL):
                        hs = slice(h * HD, (h + 1) * HD)
                        nc.vector.tensor_scalar(
                            nrm[:, hs], raws[mi][:, hs],
                            rstdg[:, mi, h:h + 1], None, op0=OP.mult)
                    tps = ps128.tile([128, 128], bf16, tag="ps128",
                                     name=f"tps{b}{int(is_q)}{mt}")
                    nc.tensor.transpose(tps, nrm, ident)
                    nc.vector.tensor_scalar(
                        dst[:, mt * 128:(mt + 1) * 128], tps, gc, bc,
                        op0=OP.mult, op1=OP.add)
                    yield

    def attn(b):
        """S^T -> exp -> (V|1)^T @ P^T, head-pair packed."""
        hoT[b] = big_pool.tile([128, N], bf16, tag="hoT", name=f"hoT{b}")
        for nc4 in range(4):  # 512-wide query column chunks
            ns = slice(nc4 * 512, (nc4 + 1) * 512)
            av = psbig.tile([128, 1024], f32, tag="av", bufs=1,
                            name=f"av{b}{nc4}")
            for mc in range(16):
                mcs = slice(mc * 128, (mc + 1) * 128)
                st = psbig.tile([128, 1024], f32, tag="st",
                                name=f"st{b}{nc4}{mc}")
                for h in range(HL):
                    nc.tensor.matmul(st[:, h * 512:(h + 1) * 512],
                                     lhsT=kT[b][h * HD:(h + 1) * HD, mcs],
                                     rhs=qT[b][h * HD:(h + 1) * HD, ns],
                                     start=True, stop=True)
                pT = pT_pool.tile([128, 1024], bf16, tag="pT")
                nc.scalar.activation(pT, st, AF.Exp)
                for h in range(HL):
                    nc.tensor.matmul(
                        av[0:HD + 1, h * 512:(h + 1) * 512],
                        lhsT=Vt[b][:, mc, h * (HD + 1):(h + 1) * (HD + 1)],
                        rhs=pT[:, h * 512:(h + 1) * 512],
                        start=(mc == 0), stop=(mc == 15),
                        skip_group_check=True)
                yield
            for h in range(HL):
                hs = slice(h * HD, (h + 1) * HD)
                sl = slice(h * 512, (h + 1) * 512)
                recip = bc_pool.tile([1, 512], f32, tag="recip")
                nc.vector.reciprocal(recip, av[HD:HD + 1, sl])
                bcast = bc_pool.tile([HD, 512], f32, tag="bcast")
                nc.gpsimd.partition_broadcast(bcast, recip)
                nc.vector.tensor_mul(hoT[b][hs, ns], av[0:HD, sl], bcast)
                yield

    def outp(b):
        """Output projection partial for batch b."""
        for nt in range(16):
            for oc in range(2):
                fps = psbig.tile([128, 512], f32, tag="st",
                                 name=f"fps{b}{nt}{oc}")
                nc.tensor.matmul(fps,
                                 lhsT=hoT[b][:, nt * 128:(nt + 1) * 128],
                                 rhs=wp_sb[:, oc * 512:(oc + 1) * 512],
                                 start=True, stop=True)
                osb = out_pool.tile([128, 512], f32, tag="osb")
                nc.any.tensor_copy(osb, fps)
                nc.gpsimd.dma_start(
                    out[b, nt * 128:(nt + 1) * 128, oc * 512:(oc + 1) * 512],
                    osb)
                yield

    def run_all(g):
        for _ in g:
            pass

    def interleave(ga, gb, ka, kb):
        """Alternate ka steps of ga with kb steps of gb until both drain."""
        alive_a, alive_b = True, True
        while alive_a or alive_b:
            for _ in range(ka):
                if alive_a:
                    alive_a = next(ga, _SENTINEL) is not _SENTINEL
            for _ in range(kb):
                if alive_b:
                    alive_b = next(gb, _SENTINEL) is not _SENTINEL

    _SENTINEL = object()

    run_all(prod(0))
    interleave(attn(0), prod(1), 1, 1)
    interleave(attn(1), outp(0), 2, 1)
    run_all(outp(1))


def _build():
    global _COMPILED
    if _COMPILED is not None:
        return _COMPILED
    import concourse.tile as tile
    from concourse import bacc, mybir

    nc = bacc.Bacc("TRN2", target_bir_lowering=False, debug=False,
                   enable_asserts=False)
    bf16 = mybir.dt.bfloat16
    f32 = mybir.dt.float32
    aps = {
        "x1": nc.dram_tensor("x1", [B, N, DM], bf16, kind="ExternalInput").ap(),
        "x2": nc.dram_tensor("x2", [B, M, DM], bf16, kind="ExternalInput").ap(),
        "wqT": nc.dram_tensor("wqT", [DM, LOC], bf16, kind="ExternalInput").ap(),
        "wkT": nc.dram_tensor("wkT", [DM, LOC], bf16, kind="ExternalInput").ap(),
        "wvT": nc.dram_tensor("wvT", [DM, LOC], bf16, kind="ExternalInput").ap(),
        "wp": nc.dram_tensor("wp", [LOC, DM], bf16, kind="ExternalInput").ap(),
        "ln_g": nc.dram_tensor("ln_g", [HD], f32, kind="ExternalInput").ap(),
        "ln_b": nc.dram_tensor("ln_b", [HD], f32, kind="ExternalInput").ap(),
        "out": nc.dram_tensor("out", [B, N, DM], f32, kind="ExternalOutput").ap(),
    }
    with tile.TileContext(nc) as tc, ExitStack() as ctx:
        _emit(ctx, tc, aps)
    nc.compile()
    _COMPILED = nc
    return nc


def kernel(x1, x2, Wq, Wk, Wv, Wp, bp, ln_g, ln_b):
    global LAST_RESULT
    from concourse.bass_utils import run_bass_kernel_spmd

    nc = _build()
    bf = ml_dtypes.bfloat16
    x1b = np.ascontiguousarray(np.asarray(x1, dtype=np.float32)).astype(bf)
    x2b = np.ascontiguousarray(np.asarray(x2, dtype=np.float32)).astype(bf)
    Wq = np.asarray(Wq, dtype=np.float32)
    Wk = np.asarray(Wk, dtype=np.float32)
    Wv = np.asarray(Wv, dtype=np.float32)
    Wp = np.asarray(Wp, dtype=np.float32)
    ln_g32 = np.ascontiguousarray(np.asarray(ln_g, dtype=np.float32))
    ln_b32 = np.ascontiguousarray(np.asarray(ln_b, dtype=np.float32))

    in_maps = []
    for c in range(NCORES):
        hs = slice(c * LOC, (c + 1) * LOC)
        in_maps.append({
            "x1": x1b,
            "x2": x2b,
            "wqT": np.ascontiguousarray(Wq[hs, :].T).astype(bf),
            "wkT": np.ascontiguousarray(Wk[hs, :].T).astype(bf),
            "wvT": np.ascontiguousarray(Wv[hs, :].T).astype(bf),
            "wp": np.ascontiguousarray(Wp[:, hs].T).astype(bf),
            "ln_g": ln_g32,
            "ln_b": ln_b32,
        })

    res = run_bass_kernel_spmd(nc, in_maps, core_ids=list(range(NCORES)))
    LAST_RESULT = res
    acc = np.zeros((B, N, DM), dtype=np.float32)
    for r in res.results:
        acc += r["out"]
    acc += np.asarray(bp, dtype=np.float32)
    return acc
